# revision 5
# baseline (speedup 1.0000x reference)
"""Trainium2 Bass kernel for 2-layer GAT (nn_GAT_45157286150549) — v4.

8-core SPMD, dst-major edge layout. vs v3b (edge-major + one-hot matmul):
- Edges laid out [partition=dst, channel=edge-rank]: the segment softmax
  denominator and the message aggregation become per-partition channel
  sums on the DVE (bf16 pair-tree + fp32 reduce) — the pm/pmt one-hot
  fp8 streams (~59MB/core) and ~1800 PE select/agg matmuls are gone.
- Nodes assigned to (core, block, partition) by Hilbert order on each
  node's (nlo, nhi) in-edge split so per-block channel counts hug the
  real degrees; pad slots gather a dedicated pad row whose stored
  attention logit is -100 (exp ~ 2e-9 -> no effect on softmax).
- The int16 gather-index halves are structural: lo nodes = orig ids
  0..32766 + one pad (row 32767 = lo pad row), hi the rest (row NTOT-1
  = hi pad row). Layers 1 and 2 share identical index streams.
"""

import os
import sys

for _p in ("/opt/trn_rl_repo", "/root/.axon_site/_ro/trn_rl_repo"):
    if os.path.isdir(_p) and _p not in sys.path:
        sys.path.insert(0, _p)

import numpy as np
import ml_dtypes

import concourse.bass as bass
import concourse.bacc as bacc
import concourse.mybir as mybir
import concourse.tile as tile
from concourse.library_config import mlp
from concourse.tile import add_dep_helper
from concourse.bass_utils import run_bass_kernel_spmd

# ---------------- problem constants ----------------
N, F_IN, E = 50000, 128, 800000
HID, HEADS, EMB = 32, 8, 64
NEG_SLOPE = 0.2

NCORES = 8
P = 128
NB = 49
NODES_PC = NB * P          # 6272
NTOT = NCORES * NODES_PC   # 50176
LO = 32768
NBLO = 32                  # blocks 0..31 hold the lo half (32*1024=32768)
TW = 384                   # layer-1 table row width (264 used, 768B)
PAD_LO_ID = NTOT - 1       # forced to mapped row 32767
PAD_HI_ID = NTOT - 2       # forced to mapped row NTOT-1

F32 = mybir.dt.float32
BF16 = mybir.dt.bfloat16
I16 = mybir.dt.int16

# chunk boundaries (block index); 32 is mandatory (the lo/hi row split)
_CHUNK_SPEC = os.environ.get("GAT_CHUNKS", "16,32,42,49")
_BNDS = [int(v) for v in _CHUNK_SPEC.split(",")]
assert _BNDS[-1] == NB and NBLO in _BNDS
CHUNKS = [(a, b) for a, b in zip([0] + _BNDS[:-1], _BNDS)]
CHUNK_ROW0 = []
_acc = 0
for _a, _b in CHUNKS:
    CHUNK_ROW0.append(_acc)
    _acc += NCORES * (_b - _a) * P
assert _acc == NTOT

SLOTB = int(os.environ.get("GAT_SLOTB", "40"))   # channels per gather group
MAX_IDX_CH = int(os.environ.get("GAT_MAX_IDX_CH", "8"))
G_BUFS = int(os.environ.get("GAT_G_BUFS", "2"))
SB_BUFS = int(os.environ.get("GAT_SB_BUFS", "2"))
REPS = int(os.environ.get("GAT_REPS", "1"))
PHASES = os.environ.get("GAT_PHASES", "abgc")
TREE_BF = int(os.environ.get("GAT_TREE_BF", "1"))

# interleave permutation: new col j <- old col (j%8)*32 + j//8
ILV = (np.arange(256) % 8) * 32 + np.arange(256) // 8


def _rank_to_row():
    """row[rank]: rank r -> core r%8, block r//1024, part (r//8)%128,
    mapped to the chunk-major row used for hA/h2A/xallT."""
    r = np.arange(NTOT)
    core = r % NCORES
    blk = r // (NCORES * P)
    part = (r // NCORES) % P
    row = np.empty(NTOT, np.int64)
    for ci, (b0, b1) in enumerate(CHUNKS):
        m = (blk >= b0) & (blk < b1)
        row[m] = (CHUNK_ROW0[ci] + core[m] * (b1 - b0) * P
                  + (blk[m] - b0) * P + part[m])
    return row


RANK_ROW = _rank_to_row()
assert (RANK_ROW[:LO] < LO).all() and (RANK_ROW[LO:] >= LO).all()


def _hilbert_d(x, y, order=7):
    x = x.astype(np.int64).copy()
    y = y.astype(np.int64).copy()
    dd = np.zeros_like(x)
    s = 1 << (order - 1)
    while s > 0:
        rx = ((x & s) > 0).astype(np.int64)
        ry = ((y & s) > 0).astype(np.int64)
        dd += s * s * ((3 * rx) ^ ry)
        swap = ry == 0
        flip = swap & (rx == 1)
        xr = np.where(flip, s - 1 - x, x)
        yr = np.where(flip, s - 1 - y, y)
        x, y = np.where(swap, yr, xr), np.where(swap, xr, yr)
        s >>= 1
    return dd


# ============================================================
# Host preprocessing
# ============================================================

def _graph_layout(src, dst):
    """Build node ranking + per-core slot structure from edges."""
    deg = np.bincount(dst, minlength=NTOT)
    lo_edge = src <= LO - 2          # srcs < 50000, so this is src ∈ S
    nlo = np.bincount(dst[lo_edge], minlength=NTOT)
    nhi = deg - nlo

    # S = ids 0..32766 + PAD_LO_ID; complement gets the rest
    ids = np.arange(NTOT)
    s_ids = np.concatenate([ids[:LO - 1], [PAD_LO_ID]])
    h_ids = np.concatenate([ids[LO - 1:N], ids[N:NTOT - 2], [PAD_HI_ID]])
    assert len(s_ids) == LO and len(h_ids) == NTOT - LO

    def order_sec(sec, force_last):
        sec = sec[sec != force_last]
        hd = _hilbert_d(np.minimum(nlo[sec], 127), np.minimum(nhi[sec], 127))
        k = np.lexsort((sec, hd))
        return np.concatenate([sec[k], [force_last]])

    order = np.concatenate([order_sec(s_ids, PAD_LO_ID),
                            order_sec(h_ids, PAD_HI_ID)])
    node_row = np.empty(NTOT, np.int64)
    node_row[order] = RANK_ROW
    rank_of = np.empty(NTOT, np.int64)
    rank_of[order] = np.arange(NTOT)
    assert node_row[PAD_LO_ID] == LO - 1 and node_row[PAD_HI_ID] == NTOT - 1

    # rank r: block r//1024; reshape order -> [NB, 1024] then max
    Llo = nlo[order].reshape(NB, NCORES * P).max(axis=1).astype(int)
    Lhi = nhi[order].reshape(NB, NCORES * P).max(axis=1).astype(int)
    Llo = np.maximum(Llo, 1)
    Lhi = np.maximum(Lhi, 1)
    return deg, nlo, nhi, lo_edge, order, node_row, rank_of, Llo, Lhi


def _make_groups(Llo, Lhi):
    groups = []
    b0 = 0
    while b0 < NB:
        b1 = b0 + 1
        tot = Llo[b0] + Lhi[b0]
        while (b1 < NB and tot + Llo[b1] + Lhi[b1] <= SLOTB
               and not (b0 < NBLO <= b1)
               and not any(b1 == e for _, e in CHUNKS)):
            tot += Llo[b1] + Lhi[b1]
            b1 += 1
        groups.append((b0, b1))
        b0 = b1
    return groups


def _idx_stream(flat_i16):
    L = len(flat_i16)
    a16 = flat_i16.reshape(L // 16, 16).T
    return np.tile(a16, (8, 1)).astype(np.int16)


def prepare(x, edge_index, W_src1, W_dst1, att_src1, att_dst1, b1,
            W_src2, W_dst2, att_src2, att_dst2, b2):
    x = np.asarray(x, np.float32)
    src = np.asarray(edge_index[0], np.int64)
    dst = np.asarray(edge_index[1], np.int64)

    att1s = np.asarray(att_src1, np.float32)
    att1d = np.asarray(att_dst1, np.float32)
    bd1s = np.zeros((HEADS * HID, HEADS), np.float32)
    bd1d = np.zeros((HEADS * HID, HEADS), np.float32)
    for h in range(HEADS):
        bd1s[h * HID:(h + 1) * HID, h] = att1s[h]
        bd1d[h * HID:(h + 1) * HID, h] = att1d[h]
    A_src1 = np.asarray(W_src1, np.float32) @ bd1s
    A_dst1 = np.asarray(W_dst1, np.float32) @ bd1d
    W1i = np.asarray(W_src1, np.float32)[:, ILV]
    W1c = np.concatenate([W1i, A_src1], axis=1)           # [128, 264]

    A_src2 = np.asarray(W_src2, np.float32) @ np.asarray(
        att_src2, np.float32).reshape(EMB, 1)
    A_dst2 = np.asarray(W_dst2, np.float32) @ np.asarray(
        att_dst2, np.float32).reshape(EMB, 1)
    W2c = np.concatenate(
        [np.asarray(W_src2, np.float32), A_src2, A_dst2], axis=1)
    W2c = W2c[ILV].reshape(2, P, 66)

    b1 = np.asarray(b1, np.float32)
    b2 = np.asarray(b2, np.float32)
    b1z = not np.any(b1)
    b2z = not np.any(b2)

    deg, nlo, nhi, lo_edge, order, node_row, rank_of, Llo, Lhi = \
        _graph_layout(src, dst)
    groups = _make_groups(Llo, Lhi)

    bf = ml_dtypes.bfloat16
    xallT = np.zeros((F_IN, NTOT), np.float32)
    xallT[:, node_row[:N]] = x.T

    # per-core node table: node id at (core, block*128+part)
    rk = np.arange(NTOT)
    core_of = rk % NCORES
    pos_of = (rk // (NCORES * P)) * P + (rk // NCORES) % P
    node_at = np.full((NCORES, NODES_PC), -1, np.int64)
    node_at[core_of, pos_of] = order

    # per-edge slot assignment
    e_src_row = node_row[src]
    e_rank = rank_of[dst]
    e_core = e_rank % NCORES
    e_pos = (e_rank // (NCORES * P)) * P + (e_rank // NCORES) % P
    CLO, CHI = int(Llo.sum()), int(Lhi.sum())
    lo_coff = np.concatenate([[0], np.cumsum(Llo)]).astype(int)
    hi_coff = np.concatenate([[0], np.cumsum(Lhi)]).astype(int)

    in_maps = []
    imaps = []
    for c in range(NCORES):
        m = e_core == c
        sr, pos, isl = e_src_row[m], e_pos[m], lo_edge[m]
        blk, prt = pos // P, pos % P
        # channel index per edge: order by (dst, side, src-row)
        okey = np.lexsort((sr, ~isl, pos))
        sr, pos, isl, blk, prt = (a[okey] for a in (sr, pos, isl, blk, prt))
        # running count per (pos, side)
        cnt_lo = np.zeros(NODES_PC, np.int64)
        cnt_hi = np.zeros(NODES_PC, np.int64)
        # edges are sorted by pos then side(lo first): use segment arithmetic
        # index within its (pos, side) segment:
        seg_key = pos * 2 + (~isl).astype(np.int64)
        seg_start = np.r_[0, np.flatnonzero(np.diff(seg_key)) + 1]
        seg_len = np.diff(np.r_[seg_start, len(seg_key)])
        within = np.arange(len(seg_key)) - np.repeat(seg_start, seg_len)

        ilv = np.full((CLO, P), LO - 1, np.int16)       # pad row 32767
        ihv = np.full((CHI, P), NTOT - 1 - LO, np.int16)  # pad row NTOT-1
        lo_m = isl
        ch_lo = lo_coff[blk[lo_m]] + within[lo_m]
        ilv[ch_lo, prt[lo_m]] = sr[lo_m].astype(np.int16)
        hi_m = ~isl
        ch_hi = hi_coff[blk[hi_m]] + within[hi_m]
        ihv[ch_hi, prt[hi_m]] = (sr[hi_m] - LO).astype(np.int16)

        il_c = _idx_stream(ilv.reshape(-1))
        ih_c = _idx_stream(ihv.reshape(-1))

        xo = np.zeros((NODES_PC, F_IN), np.float32)
        real = node_at[c] < N
        xo[real] = x[node_at[c][real]]

        in_maps.append({
            "xallT": xallT.astype(bf),
            "W1c": W1c.astype(bf),
            "Ad1": A_dst1.astype(bf),
            "W2c": W2c.astype(bf),
            "b1t": np.tile(b1[ILV][None, :], (P, 1)),
            "b2t": np.tile(b2[None, :], (P, 1)),
            "identf": np.eye(P, dtype=np.float32),
            "xownT": np.ascontiguousarray(xo.T).astype(bf),
            "il": il_c, "ih": ih_c,
        })
        imaps.append(node_at[c])
    meta = (tuple(int(v) for v in Llo), tuple(int(v) for v in Lhi),
            tuple(groups), b1z, b2z)
    return in_maps, imaps, meta


# ============================================================
# Device program
# ============================================================

def build_nc(Llo, Lhi, groups, b1z, b2z, reps=1):
    Llo, Lhi = list(Llo), list(Lhi)
    nch_b = [a + b for a, b in zip(Llo, Lhi)]
    CLO, CHI = sum(Llo), sum(Lhi)
    MAXCH = max(nch_b)

    # per-group info
    grp_info = []
    clo0 = chi0 = 0
    for b0, b1 in groups:
        nlo_g = sum(Llo[b0:b1])
        nhi_g = sum(Lhi[b0:b1])
        grp_info.append((clo0, chi0, nlo_g, nhi_g))
        clo0 += nlo_g
        chi0 += nhi_g

    nc = bacc.Bacc("TRN2", target_bir_lowering=False, debug=False,
                   num_devices=NCORES)
    dt = nc.dram_tensor
    xallT = dt("xallT", [F_IN, NTOT], BF16, kind="ExternalInput").ap()
    xownT = dt("xownT", [F_IN, NODES_PC], BF16, kind="ExternalInput").ap()
    W1c = dt("W1c", [F_IN, 264], BF16, kind="ExternalInput").ap()
    Ad1 = dt("Ad1", [F_IN, 8], BF16, kind="ExternalInput").ap()
    W2c = dt("W2c", [2, P, 66], BF16, kind="ExternalInput").ap()
    b1t = dt("b1t", [P, 256], F32, kind="ExternalInput").ap()
    b2t = dt("b2t", [P, 64], F32, kind="ExternalInput").ap()
    identf = dt("identf", [P, P], F32, kind="ExternalInput").ap()
    ilD = dt("il", [P, CLO * 8], I16, kind="ExternalInput").ap()
    ihD = dt("ih", [P, CHI * 8], I16, kind="ExternalInput").ap()

    out2 = dt("out2", [NODES_PC, EMB], F32, kind="ExternalOutput").ap()

    hA = dt("hA", [NTOT, TW], BF16).ap()
    cc_cs = [dt(f"cc_in{i}", [(b1 - b0) * P, P], BF16).ap()
             for i, (b0, b1) in enumerate(CHUNKS)]
    h2A = dt("h2A", [NTOT, P], BF16, addr_space="Shared").ap()

    with tile.TileContext(nc) as tc:
        with (
            tc.tile_pool(name="const", bufs=1) as cp,
            tc.tile_pool(name="persist", bufs=1) as pp,
        ):
            lib_inst = nc.gpsimd.load_library(mlp)

            def gather(**kw):
                g = nc.gpsimd.dma_gather(**kw)
                add_dep_helper(g.ins, lib_inst.ins, sync=True,
                               reason="mlp library before gather")
                return g

            w1_sb = cp.tile([F_IN, 264], BF16)
            nc.sync.dma_start(out=w1_sb[:], in_=W1c[:])
            ad1_sb = cp.tile([F_IN, 8], BF16)
            nc.sync.dma_start(out=ad1_sb[:], in_=Ad1[:])
            w2a_sb = cp.tile([P, 66], BF16, tag="w2a")
            nc.sync.dma_start(out=w2a_sb[:], in_=W2c[0])
            w2b_sb = cp.tile([P, 66], BF16, tag="w2b")
            nc.sync.dma_start(out=w2b_sb[:], in_=W2c[1])
            identf_sb = cp.tile([P, P], F32, tag="identf")
            nc.sync.dma_start(out=identf_sb[:], in_=identf[:])
            if not b1z:
                b1_sb = cp.tile([P, 256], F32)
                nc.sync.dma_start(out=b1_sb[:], in_=b1t[:])
            if not b2z:
                b2_sb = cp.tile([P, 64], F32)
                nc.sync.dma_start(out=b2_sb[:], in_=b2t[:])
            ilr = cp.tile([P, CLO * 8], I16, tag="ilr")
            nc.sync.dma_start(out=ilr[:], in_=ilD[:])
            ihr = cp.tile([P, CHI * 8], I16, tag="ihr")
            nc.sync.dma_start(out=ihr[:], in_=ihD[:])
            padv = cp.tile([1, 8], BF16, tag="padv")
            nc.vector.memset(padv[:], -100.0)

            a2bf = pp.tile([P, NB], BF16)

            LREL = mybir.ActivationFunctionType.Prelu
            EXPF = mybir.ActivationFunctionType.Exp

            for rep in range(reps):
                # ---------- Phase A: layer-1 node table ----------
                with (
                    tc.tile_pool(name=f"pa_sb{rep}", bufs=3) as pa,
                    tc.tile_pool(name=f"pa_ps{rep}", bufs=2, space="PSUM") as paps,
                ):
                    for m0 in range(0, NTOT // P if "a" in PHASES else 0, 2):
                        xs = pa.tile([F_IN, 2 * P], BF16, tag="xs")
                        nc.sync.dma_start(
                            out=xs[:], in_=xallT[:, m0 * P:(m0 + 2) * P])
                        psA = paps.tile([P, 2, 512], F32, tag="psA")
                        for k in range(2):
                            nc.tensor.matmul(
                                psA[:, k, 0:264],
                                lhsT=xs[:, k * P:(k + 1) * P],
                                rhs=w1_sb[:], start=True, stop=True)
                        hbf = pa.tile([P, 2, 264], BF16, tag="hbf")
                        nc.scalar.copy(out=hbf[:], in_=psA[:, :, 0:264])
                        dst_rows = bass.AP(
                            hA.tensor, m0 * P * TW,
                            [[TW, P], [TW * P, 2], [1, 264]])
                        nc.gpsimd.dma_start(out=dst_rows, in_=hbf[:])
                    if "a" in PHASES:
                        # pad rows: a_src := -100 (h stays 0)
                        nc.sync.dma_start(
                            out=hA[LO - 1:LO, 256:264], in_=padv[:])
                        nc.sync.dma_start(
                            out=hA[NTOT - 1:NTOT, 256:264], in_=padv[:])

                # ---------- Phase B: layer-1 edge pass ----------
                with (
                    tc.tile_pool(name=f"pb_g{rep}", bufs=G_BUFS) as pg,
                    tc.tile_pool(name=f"pb_sb{rep}", bufs=SB_BUFS) as pb,
                    tc.tile_pool(name=f"pb_exm{rep}", bufs=SB_BUFS) as px,
                    tc.tile_pool(name=f"pb_ps{rep}", bufs=2, space="PSUM") as pps,
                ):
                    for gi, (b0, b1) in enumerate(
                            groups if "b" in PHASES else []):
                        clo0, chi0, nlo_g, nhi_g = grp_info[gi]
                        nch_g = nlo_g + nhi_g
                        xo = pb.tile([F_IN, (b1 - b0) * P], BF16, tag="xo")
                        nc.sync.dma_start(
                            out=xo[:], in_=xownT[:, b0 * P:b1 * P])
                        G = pg.tile([P, nch_g, TW], BF16, tag="G")
                        for nch_s, idxt, i0, tab, coff in (
                                (nlo_g, ilr, clo0, hA[0:LO, :], 0),
                                (nhi_g, ihr, chi0, hA[LO:NTOT, :], nlo_g)):
                            for c0 in range(0, nch_s, MAX_IDX_CH):
                                cn = min(MAX_IDX_CH, nch_s - c0)
                                gather(
                                    out_ap=G[:, coff + c0:coff + c0 + cn, :],
                                    in_ap=tab,
                                    idxs_ap=idxt[:, (i0 + c0) * 8:
                                                 (i0 + c0 + cn) * 8],
                                    num_idxs=cn * P, num_idxs_reg=cn * P,
                                    elem_size=TW)

                        lo_off = 0
                        hi_off = nlo_g
                        for b in range(b0, b1):
                            nlo_b, nhi_b = Llo[b], Lhi[b]
                            nch = nlo_b + nhi_b
                            slo = slice(lo_off, lo_off + nlo_b)
                            shi = slice(hi_off, hi_off + nhi_b)
                            lo_off += nlo_b
                            hi_off += nhi_b

                            pa1 = pps.tile([P, 8], F32, tag="pa1")
                            nc.tensor.matmul(
                                pa1[:],
                                lhsT=xo[:, (b - b0) * P:(b - b0 + 1) * P],
                                rhs=ad1_sb[:], start=True, stop=True)
                            a1 = pb.tile([P, 8], BF16, tag="a1")
                            nc.scalar.copy(out=a1[:], in_=pa1[:])
                            a1b = a1[:].rearrange(
                                "p (c h) -> p c h", c=1)

                            ee = pb.tile([P, nch, 8], BF16, tag="ee")
                            nc.vector.tensor_tensor(
                                out=ee[:, 0:nlo_b, :],
                                in0=G[:, slo, 256:264],
                                in1=a1b.to_broadcast([P, nlo_b, 8]),
                                op=mybir.AluOpType.add)
                            nc.vector.tensor_tensor(
                                out=ee[:, nlo_b:nch, :],
                                in0=G[:, shi, 256:264],
                                in1=a1b.to_broadcast([P, nhi_b, 8]),
                                op=mybir.AluOpType.add)
                            e2 = pb.tile([P, nch, 8], BF16, tag="e2")
                            nc.scalar.activation(
                                out=e2[:], in_=ee[:], func=LREL,
                                alpha=NEG_SLOPE)
                            exm = px.tile([P, MAXCH, 264], BF16, tag="exm")
                            nc.scalar.activation(
                                out=exm[:, 0:nch, 0:8], in_=e2[:],
                                func=EXPF)
                            for sec, gsec, o0 in ((slice(0, nlo_b), slo, 0),
                                                  (slice(nlo_b, nch), shi,
                                                   nlo_b)):
                                nn = sec.stop - sec.start
                                nc.vector.tensor_tensor(
                                    out=exm[:, sec, 8:264].rearrange(
                                        "p b (c h) -> p b c h", h=8),
                                    in0=G[:, gsec, 0:256].rearrange(
                                        "p b (c h) -> p b c h", h=8),
                                    in1=exm[:, o0:o0 + nn, 0:8].rearrange(
                                        "p b (c h) -> p b c h", c=1
                                    ).to_broadcast([P, nn, 32, 8]),
                                    op=mybir.AluOpType.mult)

                            # channel sum: bf16 pair level(s), fp32 reduce
                            red_in = exm[:]
                            nred = nch
                            for _lvl in range(TREE_BF):
                                if nred < 4:
                                    break
                                half = nred // 2
                                tb_ = px.tile([P, (MAXCH + 1) // 2, 264],
                                              BF16, tag=f"tb{_lvl}")
                                nc.vector.tensor_tensor(
                                    out=tb_[:, 0:half, :],
                                    in0=red_in[:, 0:half, :],
                                    in1=red_in[:, half:2 * half, :],
                                    op=mybir.AluOpType.add)
                                if nred % 2:
                                    nc.scalar.copy(
                                        out=tb_[:, half:half + 1, :],
                                        in_=red_in[:, 2 * half:nred, :])
                                    nred = half + 1
                                else:
                                    nred = half
                                red_in = tb_[:]
                            agg = pb.tile([P, 264], F32, tag="agg")
                            nc.vector.tensor_reduce(
                                out=agg[:],
                                in_=red_in[:, 0:nred, :].rearrange(
                                    "p c e -> p e c"),
                                axis=mybir.AxisListType.X,
                                op=mybir.AluOpType.add)

                            R = pb.tile([P, 8], F32, tag="R")
                            nc.vector.reciprocal(R[:], agg[:, 0:8])
                            h1 = pb.tile([P, 256], F32, tag="h1")
                            nc.vector.tensor_tensor(
                                out=h1[:].rearrange("p (c h) -> p c h", h=8),
                                in0=agg[:, 8:264].rearrange(
                                    "p (c h) -> p c h", h=8),
                                in1=R[:].rearrange("p (c h) -> p c h", c=1)
                                .to_broadcast([P, 32, 8]),
                                op=mybir.AluOpType.mult)
                            if not b1z:
                                nc.vector.tensor_add(out=h1[:], in0=h1[:],
                                                     in1=b1_sb[:])
                            nc.vector.tensor_scalar_max(h1[:], h1[:], 0.0)

                            ps2 = pps.tile([P, 66], F32, tag="ps2")
                            for k in range(2):
                                tp = pps.tile([P, P], F32, tag="tp")
                                nc.tensor.transpose(
                                    out=tp[:], in_=h1[:, k * P:(k + 1) * P],
                                    identity=identf_sb[:])
                                hT = pb.tile([P, P], BF16, tag="hT")
                                nc.scalar.copy(out=hT[:], in_=tp[:])
                                nc.tensor.matmul(
                                    ps2[:], lhsT=hT[:],
                                    rhs=(w2a_sb[:] if k == 0 else w2b_sb[:]),
                                    start=(k == 0), stop=(k == 1))
                            cbf = pb.tile([P, 65], BF16, tag="cbf")
                            nc.scalar.copy(out=cbf[:], in_=ps2[:, 0:65])
                            nc.scalar.copy(out=a2bf[:, b:b + 1],
                                           in_=ps2[:, 65:66])
                            ci = next(i for i, (c0_, c1_) in enumerate(CHUNKS)
                                      if c0_ <= b < c1_)
                            cb0 = CHUNKS[ci][0]
                            nc.sync.dma_start(
                                out=cc_cs[ci][(b - cb0) * P:
                                              (b - cb0 + 1) * P, 0:65],
                                in_=cbf[:])

                        if "g" in PHASES:
                            for ci, (cb0, cb1) in enumerate(CHUNKS):
                                if b1 == cb1:
                                    r0 = CHUNK_ROW0[ci]
                                    nrow = NCORES * (cb1 - cb0) * P
                                    nc.gpsimd.collective_compute(
                                        "AllGather", mybir.AluOpType.bypass,
                                        replica_groups=[list(range(NCORES))],
                                        ins=[cc_cs[ci][:].opt()],
                                        outs=[h2A[r0:r0 + nrow, :].opt()])
                                    if cb1 == NBLO:
                                        nc.sync.dma_start(
                                            out=h2A[LO - 1:LO, 64:65],
                                            in_=padv[:, 0:1])
                                    if cb1 == NB:
                                        nc.sync.dma_start(
                                            out=h2A[NTOT - 1:NTOT, 64:65],
                                            in_=padv[:, 0:1])

                # ---------- Phase C: layer-2 edge pass ----------
                with (
                    tc.tile_pool(name=f"pc_g{rep}", bufs=G_BUFS) as pg2,
                    tc.tile_pool(name=f"pc_sb{rep}", bufs=SB_BUFS) as pc,
                    tc.tile_pool(name=f"pc_exm{rep}", bufs=SB_BUFS) as px2,
                ):
                    for gi, (b0, b1) in enumerate(
                            groups if "c" in PHASES else []):
                        clo0, chi0, nlo_g, nhi_g = grp_info[gi]
                        nch_g = nlo_g + nhi_g
                        G2 = pg2.tile([P, nch_g, P], BF16, tag="G2")
                        for nch_s, idxt, i0, tab, coff in (
                                (nlo_g, ilr, clo0, h2A[0:LO, :], 0),
                                (nhi_g, ihr, chi0, h2A[LO:NTOT, :], nlo_g)):
                            for c0 in range(0, nch_s, MAX_IDX_CH):
                                cn = min(MAX_IDX_CH, nch_s - c0)
                                gather(
                                    out_ap=G2[:, coff + c0:coff + c0 + cn, :],
                                    in_ap=tab,
                                    idxs_ap=idxt[:, (i0 + c0) * 8:
                                                 (i0 + c0 + cn) * 8],
                                    num_idxs=cn * P, num_idxs_reg=cn * P,
                                    elem_size=P)

                        lo_off = 0
                        hi_off = nlo_g
                        for b in range(b0, b1):
                            nlo_b, nhi_b = Llo[b], Lhi[b]
                            nch = nlo_b + nhi_b
                            slo = slice(lo_off, lo_off + nlo_b)
                            shi = slice(hi_off, hi_off + nhi_b)
                            lo_off += nlo_b
                            hi_off += nhi_b

                            a2b = a2bf[:, b:b + 1].rearrange(
                                "p (c h) -> p c h", c=1)
                            ec = pc.tile([P, nch, 1], BF16, tag="ec")
                            nc.vector.tensor_tensor(
                                out=ec[:, 0:nlo_b, :],
                                in0=G2[:, slo, 64:65],
                                in1=a2b.to_broadcast([P, nlo_b, 1]),
                                op=mybir.AluOpType.add)
                            nc.vector.tensor_tensor(
                                out=ec[:, nlo_b:nch, :],
                                in0=G2[:, shi, 64:65],
                                in1=a2b.to_broadcast([P, nhi_b, 1]),
                                op=mybir.AluOpType.add)
                            lr1 = pc.tile([P, nch, 1], BF16, tag="lr1")
                            nc.scalar.activation(
                                out=lr1[:], in_=ec[:], func=LREL,
                                alpha=NEG_SLOPE)
                            ex8 = pc.tile([P, nch, 8], BF16, tag="ex8")
                            nc.scalar.activation(
                                out=ex8[:],
                                in_=lr1[:].to_broadcast([P, nch, 8]),
                                func=EXPF)
                            exm2 = px2.tile([P, MAXCH, 64], BF16, tag="exm2")
                            for sec, gsec, o0 in ((slice(0, nlo_b), slo, 0),
                                                  (slice(nlo_b, nch), shi,
                                                   nlo_b)):
                                nn = sec.stop - sec.start
                                nc.vector.tensor_tensor(
                                    out=exm2[:, sec, :].rearrange(
                                        "p b (c h) -> p b c h", h=8),
                                    in0=G2[:, gsec, 0:64].rearrange(
                                        "p b (c h) -> p b c h", h=8),
                                    in1=ex8[:, o0:o0 + nn, :].rearrange(
                                        "p b (c h) -> p b c h", c=1
                                    ).to_broadcast([P, nn, 8, 8]),
                                    op=mybir.AluOpType.mult)

                            red_in = exm2[:]
                            nred = nch
                            for _lvl in range(TREE_BF):
                                if nred < 4:
                                    break
                                half = nred // 2
                                tb_ = px2.tile([P, (MAXCH + 1) // 2, 64],
                                               BF16, tag=f"tc{_lvl}")
                                nc.vector.tensor_tensor(
                                    out=tb_[:, 0:half, :],
                                    in0=red_in[:, 0:half, :],
                                    in1=red_in[:, half:2 * half, :],
                                    op=mybir.AluOpType.add)
                                if nred % 2:
                                    nc.scalar.copy(
                                        out=tb_[:, half:half + 1, :],
                                        in_=red_in[:, 2 * half:nred, :])
                                    nred = half + 1
                                else:
                                    nred = half
                                red_in = tb_[:]
                            agg2 = pc.tile([P, 64], F32, tag="agg2")
                            nc.vector.tensor_reduce(
                                out=agg2[:],
                                in_=red_in[:, 0:nred, :].rearrange(
                                    "p c e -> p e c"),
                                axis=mybir.AxisListType.X,
                                op=mybir.AluOpType.add)
                            den2 = pc.tile([P, 1], F32, tag="den2")
                            nc.vector.tensor_reduce(
                                out=den2[:],
                                in_=ex8[:, :, 0:1].rearrange(
                                    "p c e -> p e c"),
                                axis=mybir.AxisListType.X,
                                op=mybir.AluOpType.add)
                            R2 = pc.tile([P, 1], F32, tag="R2")
                            nc.vector.reciprocal(R2[:], den2[:])
                            o2 = pc.tile([P, 64], F32, tag="o2")
                            nc.vector.tensor_scalar(
                                out=o2[:], in0=agg2[:],
                                scalar1=R2[:], scalar2=None,
                                op0=mybir.AluOpType.mult)
                            if not b2z:
                                nc.vector.tensor_add(out=o2[:], in0=o2[:],
                                                     in1=b2_sb[:])
                            nc.sync.dma_start(
                                out=out2[b * P:(b + 1) * P, :], in_=o2[:])

    nc.compile()
    return nc


_NC_CACHE = {}


def kernel(**inputs) -> np.ndarray:
    in_maps, imaps, meta = prepare(**inputs)
    key = (meta, REPS, PHASES, MAX_IDX_CH, SLOTB, TREE_BF)
    if key not in _NC_CACHE:
        _NC_CACHE[key] = build_nc(*meta, reps=REPS)
    nc = _NC_CACHE[key]
    res = run_bass_kernel_spmd(
        nc, in_maps, core_ids=list(range(NCORES)),
        trace=bool(int(os.environ.get("GAT_TRACE", "0"))))
    kernel.last_results = res
    out = np.zeros((NTOT, EMB), np.float32)
    for c in range(NCORES):
        out[imaps[c]] = res.results[c]["out2"]
    return out[:N].astype(np.float32)
